# revision 11
# baseline (speedup 1.0000x reference)
"""Trainium2 Bass kernel for nn_ClusteringLayer (vq_codebook).

Computes, for x (B,D) and clusters (K,D):
    sq   = ||x_i||^2 - 2 x.clusters^T + ||c_j||^2     (B,K)
    out  = (1+sqrt(sq))^-1 / sum(...)                 global normalizer

Key algebra: out = 1/(S*(1+dist)) = rsqrt(S^2*(1+sqrt(sq))^2), and over the
narrow empirical dist range (~[27,38]) the square (1+sqrt(v))^2 is affine in
v to ~5e-4 rel: (1+sqrt(v))^2 ~ A*v + C.  So the pointwise tail is a single
scalar-engine op per element: out = Abs_reciprocal_sqrt(scale * w) writing
final bf16, with w = psum + cc_p + xx_b assembled by ONE DVE
scalar_tensor_tensor per k-tile ((psum add cc) add xx: the per-partition
scalar slot carries c2 and all constants, in1 carries the centered-x2 row
broadcast).  Tensor engine runs ONLY the 64 uniform fp8e4 DoubleRow matmuls
per core - microbenchmarks showed mixing bf16 K=1 fold matmuls into the DR
stream costs ~300ns/switch in PE mode churn (541 vs 229 ns/matmul).

Host precomputes x2/c2 (exact f32), the global normalizer S from a 2048-row
strided sample (SE ~2e-4), and the affine fit.  Device layout is transposed
(K on partitions, batch on free).  Whole-pipeline host sim incl fp8
quantization and bf16 output rounding: max rel err 8.4e-3 (gate 2e-2);
measured on HW: 8.4e-3 (the ARS table adds nothing measurable).

Per-core device program (8 k-tiles of 128 partitions, 2048 batch columns):
  - DMA in cc|scale (128,9) f32, ct (128,8,4,128) fp8, xt (128,4,2048) fp8
    in column quarters, x2 row broadcast (128,2048) bf16
  - per k-tile: 8 fp8e4 DR matmuls into a (128,2048) 4-bank PSUM tile,
    depth-2 pipeline; DVE (psum+cc)+xx in place; ACT ARS -> final bf16
  - 4 output DMAs of 2 k-tiles (1 MB, contiguous 8KB/partition)
Host unshards by transposing each core's (1024,2048) slab.
"""

import numpy as np

B, D, K = 16384, 512, 1024
N_CORES = 8
BL = B // N_CORES        # 2048 batch columns per core
P = 128                  # partitions
KT = K // P              # 8 k-tiles per core
KS = D // P              # 4 contraction chunks of 128
NQ = BL // 512           # 4 psum-bank quarters per tile
PIPE = 2                 # psum tiles in flight (4 banks each)

_CACHE = {}


def _build_bass():
    import concourse.bass as bass  # noqa: F401
    import concourse.mybir as mybir
    import concourse.tile as tile
    from concourse import bacc

    f32 = mybir.dt.float32
    bf16 = mybir.dt.bfloat16
    fp8 = mybir.dt.float8e4
    AF = mybir.ActivationFunctionType
    ALU = mybir.AluOpType
    DR = mybir.MatmulPerfMode.DoubleRow

    nc = bacc.Bacc(
        "TRN2", target_bir_lowering=False, debug=False, num_devices=N_CORES
    )
    xt_d = nc.dram_tensor("xt", [P, KS * BL], fp8, kind="ExternalInput").ap()
    ct_d = nc.dram_tensor("ct", [P, KT * KS * P], fp8, kind="ExternalInput").ap()
    xx_d = nc.dram_tensor("xx", [P, BL], bf16, kind="ExternalInput").ap()
    xr_d = nc.dram_tensor("xr", [1, BL], bf16, kind="ExternalInput").ap()
    cs_d = nc.dram_tensor("cs", [P, KT + 2], f32, kind="ExternalInput").ap()
    out_d = nc.dram_tensor("out", [K, BL], bf16, kind="ExternalOutput").ap()

    # k-tiles whose x2 fold runs on the tensor engine (K=1 bf16 matmuls,
    # cc absorbed into the ACT bias) instead of DVE; balances tensor
    # ~16.3us / DVE ~16.5us / ACT ~16.5us busy.
    TFOLD = (0,)

    with tile.TileContext(nc) as tc:
        with (
            tc.tile_pool(name="const", bufs=1) as cpool,
            tc.tile_pool(name="big", bufs=1) as bpool,
            tc.tile_pool(name="wst", bufs=PIPE) as wpool,
            tc.tile_pool(name="pmm", bufs=PIPE, space="PSUM") as pmm,
        ):
            csv = cpool.tile([P, KT + 2], f32)   # cc 0..7 | scale | bias(t0)
            nc.sync.dma_start(csv, cs_d)
            ones_rowb = cpool.tile([1, P], bf16)  # lhsT for x2 fold bcast
            nc.gpsimd.memset(ones_rowb, 1.0)
            xr = cpool.tile([1, BL], bf16)        # x2 fold row (early, small)
            nc.gpsimd.dma_start(xr, xr_d)

            # Input DMA issues spread across idle queues; transfers ordered
            # by first need (tile0 quarters -> ct rest -> xx for the DVE).
            ct = bpool.tile([P, KT, KS, P], fp8, name="ct")
            ct_dv = ct_d.rearrange("p (t s j) -> p t s j", t=KT, s=KS)
            nc.sync.dma_start(ct[:, 0:1], ct_dv[:, 0:1])
            xt = bpool.tile([P, KS, BL], fp8, name="xt")
            xt_dv = xt_d.rearrange("p (s b) -> p s b", s=KS)
            for q in range(2):
                nc.sync.dma_start(
                    xt[:, :, q * 512 : (q + 1) * 512],
                    xt_dv[:, :, q * 512 : (q + 1) * 512],
                )
            for q in range(2, NQ):
                nc.scalar.dma_start(
                    xt[:, :, q * 512 : (q + 1) * 512],
                    xt_dv[:, :, q * 512 : (q + 1) * 512],
                )
            nc.gpsimd.dma_start(ct[:, 1:KT], ct_dv[:, 1:KT])
            xx = bpool.tile([P, BL], bf16, name="xx")
            nc.gpsimd.dma_start(xx, xx_d)

            numbuf = bpool.tile([P, KT * BL], bf16)  # 32 KB/partition

            # ACT table prefetch: 1-elem ARS forces the lazy table load
            # during the DMA window instead of before the first real tile.
            tblw = cpool.tile([1, 1], f32)
            nc.scalar.activation(
                tblw, csv[0:1, 0:1], AF.Abs_reciprocal_sqrt, bias=0.0,
                scale=1.0,
            )

            def tile_mm(kt, ps=None):
                if ps is None:
                    ps = pmm.tile([P, BL], f32, tag="mm")
                for q in range(NQ):
                    psl = ps[:, q * 512 : (q + 1) * 512]
                    for s in range(KS // 2):
                        nc.tensor.matmul(
                            psl,
                            lhsT=ct[:, kt, 2 * s : 2 * s + 2, :],
                            rhs=xt[:, 2 * s : 2 * s + 2, q * 512 : (q + 1) * 512],
                            start=(s == 0),
                            stop=(s == KS // 2 - 1 and kt not in TFOLD),
                            perf_mode=DR,
                        )
                if kt in TFOLD:  # batched folds: one fp8->bf16 mode switch
                    for q in range(NQ):
                        nc.tensor.matmul(
                            ps[:, q * 512 : (q + 1) * 512],
                            lhsT=ones_rowb,
                            rhs=xr[0:1, q * 512 : (q + 1) * 512],
                            start=False,
                            stop=True,
                            skip_group_check=True,
                        )
                return ps

            def tile_fix(kt, ps):
                # w = (psum + cc_kt) + xx into SBUF, freeing psum for the PE
                w = wpool.tile([P, BL], f32, tag="w", name="w")
                nc.vector.scalar_tensor_tensor(
                    w, ps, csv[:, kt : kt + 1], xx, ALU.add, ALU.add
                )
                return w

            def tile_act(kt, src, bias):
                nc.scalar.activation(
                    numbuf[:, kt * BL : (kt + 1) * BL], src,
                    AF.Abs_reciprocal_sqrt,
                    bias=bias, scale=csv[:, KT : KT + 1],
                )

            def tile_store(kt):  # one k-tile per DMA (512 KB bf16)
                nc.sync.dma_start(
                    out_d[kt * P : (kt + 1) * P, :],
                    numbuf[:, kt * BL : (kt + 1) * BL],
                )

            # PE warmup: ~14 garbage matmuls (gated only on the early xr DMA)
            # keep the PE busy through the input-DMA window so DVFS ramps to
            # max clock before the real stream; the first real matmul
            # overwrites the bank with start=True.
            ps0 = pmm.tile([P, BL], f32, tag="mm")
            for i in range(14):
                nc.tensor.matmul(
                    ps0[:, 0:512],
                    lhsT=ones_rowb,
                    rhs=xr[0:1, 0:512],
                    start=True,
                    stop=True,
                    skip_group_check=True,
                )

            pend = {}
            pend[0] = tile_mm(0, ps0)
            for kt in range(1, PIPE):
                pend[kt] = tile_mm(kt)
            for kt in range(KT):
                if kt + PIPE < KT:
                    pend[kt + PIPE] = tile_mm(kt + PIPE)
                ps = pend.pop(kt)
                if kt in TFOLD:
                    tile_act(kt, ps, csv[:, KT + 1 : KT + 2])
                else:
                    w = tile_fix(kt, ps)
                    tile_act(kt, w, 0.0)
                tile_store(kt)

    nc.finalize()
    return nc


def _get_bass():
    if "nc" not in _CACHE:
        _CACHE["nc"] = _build_bass()
    return _CACHE["nc"]


def _host_prep(x: np.ndarray, clusters: np.ndarray):
    import ml_dtypes

    x = np.ascontiguousarray(x, dtype=np.float32)
    c = np.ascontiguousarray(clusters, dtype=np.float32)
    x2 = np.einsum("bd,bd->b", x, x, optimize=True)          # (B,)
    c2 = np.einsum("kd,kd->k", c, c, optimize=True)          # (K,)
    x2m = float(x2.mean())

    # Normalizer estimate from a strided 2048-row sample (SE ~2e-4 rel).
    xs = x[::8]
    sq_s = x2[::8][:, None] - 2.0 * (xs @ c.T) + c2[None, :]
    dist_s = np.sqrt(np.maximum(sq_s, 0.0))
    S = float((1.0 / (1.0 + dist_s)).sum() * (B / xs.shape[0]))

    # Affine fit (1+sqrt(v))^2 ~ A*v + C, rel-weighted LSQ over padded range
    vmin = max(float(sq_s.min()) * 0.96, 1.0)
    vmax = float(sq_s.max()) * 1.04
    v = np.linspace(vmin, vmax, 4001)
    g = (1.0 + np.sqrt(v)) ** 2
    M = np.stack([v / g, 1.0 / g], axis=1)
    (A, C), *_ = np.linalg.lstsq(M, np.ones_like(v), rcond=None)
    h = (A * v + C) / g - 1.0
    corr = 1.0 + (h.max() + h.min()) / 2.0   # recenter the error band
    A, C = float(A / corr), float(C / corr)

    S2A = S * S * A
    # ACT input u = scale*w, w = psum + cc_p + xx_b, u = S2A*sq + S2*C:
    #   scale = -2*S2A; cc_p = -(c2_p + x2m + C/A)/2; xx_b = -(x2_b - x2m)/2
    # Tensor-fold tiles skip the DVE cc add; their ACT bias = scale*cc.
    cs = np.empty((P, KT + 2), dtype=np.float32)
    cs[:, :KT] = -(c2.reshape(KT, P).T + x2m + C / A) / 2.0
    cs[:, KT] = -2.0 * S2A
    cs[:, KT + 1] = cs[:, KT] * cs[:, 0]

    ct8 = np.ascontiguousarray(
        c.reshape(KT, P, KS, P).transpose(3, 0, 2, 1).reshape(P, KT * KS * P)
    ).astype(ml_dtypes.float8_e4m3)

    in_maps = []
    for ci in range(N_CORES):
        xc = x[ci * BL : (ci + 1) * BL]
        xt8 = np.ascontiguousarray(
            xc.T.reshape(KS, P, BL).transpose(1, 0, 2).reshape(P, KS * BL)
        ).astype(ml_dtypes.float8_e4m3)
        xxrow = (-(x2[ci * BL : (ci + 1) * BL] - x2m) / 2.0).astype(
            ml_dtypes.bfloat16
        )
        xx = np.ascontiguousarray(np.broadcast_to(xxrow[None, :], (P, BL)))
        in_maps.append({
            "xt": xt8, "ct": ct8, "xx": xx,
            "xr": np.ascontiguousarray(xxrow.reshape(1, BL)), "cs": cs,
        })
    return in_maps


def _assemble(results) -> np.ndarray:
    out = np.empty((B, K), dtype=np.float32)
    for ci, r in enumerate(results):
        slab = np.asarray(r["out"]).astype(np.float32)   # (K, BL)
        out[ci * BL : (ci + 1) * BL, :] = slab.T
    return out


def kernel(x: np.ndarray, clusters: np.ndarray) -> np.ndarray:
    from concourse.bass_utils import run_bass_kernel_spmd

    x = np.asarray(x, dtype=np.float32)
    clusters = np.asarray(clusters, dtype=np.float32)
    assert x.shape == (B, D) and clusters.shape == (K, D)

    in_maps = _host_prep(x, clusters)
    nc = _get_bass()
    res = run_bass_kernel_spmd(nc, in_maps, core_ids=list(range(N_CORES)))
    return _assemble(res.results)


# revision 13
# speedup vs baseline: 1.0316x; 1.0316x over previous
"""Trainium2 Bass kernel for nn_ClusteringLayer (vq_codebook).

Computes, for x (B,D) and clusters (K,D):
    sq   = ||x_i||^2 - 2 x.clusters^T + ||c_j||^2     (B,K)
    out  = (1+sqrt(sq))^-1 / sum(...)                 global normalizer

Key algebra: out = 1/(S*(1+dist)) = rsqrt(S^2*(1+sqrt(sq))^2), and over the
narrow empirical dist range (~[27,38]) the square (1+sqrt(v))^2 is affine in
v to ~5e-4 rel: (1+sqrt(v))^2 ~ A*v + C.  So the pointwise tail is a single
scalar-engine op per element: out = Abs_reciprocal_sqrt(scale * w) writing
final bf16, with w = psum + cc_p + xx_b assembled by ONE DVE
scalar_tensor_tensor per k-tile ((psum add cc) add xx: the per-partition
scalar slot carries c2 and all constants, in1 carries the centered-x2 row
broadcast).  Tensor engine runs ONLY the 64 uniform fp8e4 DoubleRow matmuls
per core - microbenchmarks showed mixing bf16 K=1 fold matmuls into the DR
stream costs ~300ns/switch in PE mode churn (541 vs 229 ns/matmul).

Host precomputes x2/c2 (exact f32), the global normalizer S from a 2048-row
strided sample (SE ~2e-4), and the affine fit.  Device layout is transposed
(K on partitions, batch on free).  Whole-pipeline host sim incl fp8
quantization and bf16 output rounding: max rel err 8.4e-3 (gate 2e-2);
measured on HW: 8.4e-3 (the ARS table adds nothing measurable).

Per-core device program (8 k-tiles of 128 partitions, 2048 batch columns):
  - DMA in cc|scale (128,9) f32, ct (128,8,4,128) fp8, xt (128,4,2048) fp8
    in column quarters, x2 row broadcast (128,2048) bf16
  - per k-tile: 8 fp8e4 DR matmuls into a (128,2048) 4-bank PSUM tile,
    depth-2 pipeline; DVE (psum+cc)+xx in place; ACT ARS -> final bf16
  - 4 output DMAs of 2 k-tiles (1 MB, contiguous 8KB/partition)
Host unshards by transposing each core's (1024,2048) slab.
"""

import numpy as np

B, D, K = 16384, 512, 1024
N_CORES = 8
BL = B // N_CORES        # 2048 batch columns per core
P = 128                  # partitions
KT = K // P              # 8 k-tiles per core
KS = D // P              # 4 contraction chunks of 128
NQ = BL // 512           # 4 psum-bank quarters per tile
PIPE = 2                 # psum tiles in flight (4 banks each)

_CACHE = {}


def _build_bass():
    import concourse.bass as bass  # noqa: F401
    import concourse.mybir as mybir
    import concourse.tile as tile
    from concourse import bacc

    f32 = mybir.dt.float32
    bf16 = mybir.dt.bfloat16
    fp8 = mybir.dt.float8e4
    AF = mybir.ActivationFunctionType
    ALU = mybir.AluOpType
    DR = mybir.MatmulPerfMode.DoubleRow

    nc = bacc.Bacc(
        "TRN2", target_bir_lowering=False, debug=False, num_devices=N_CORES
    )
    xt_d = nc.dram_tensor("xt", [P, KS * BL], fp8, kind="ExternalInput").ap()
    ct_d = nc.dram_tensor("ct", [P, KT * KS * P], fp8, kind="ExternalInput").ap()
    xr_d = nc.dram_tensor("xr", [1, BL], bf16, kind="ExternalInput").ap()
    cs_d = nc.dram_tensor("cs", [P, KT + 2], f32, kind="ExternalInput").ap()
    out_d = nc.dram_tensor("out", [K, BL], bf16, kind="ExternalOutput").ap()

    # k-tiles whose x2 fold runs on the tensor engine (K=1 bf16 matmuls,
    # cc absorbed into the ACT bias) instead of DVE; balances tensor
    # ~16.3us / DVE ~16.5us / ACT ~16.5us busy.
    TFOLD = (0,)

    with tile.TileContext(nc) as tc:
        with (
            tc.tile_pool(name="const", bufs=1) as cpool,
            tc.tile_pool(name="big", bufs=1) as bpool,
            tc.tile_pool(name="wst", bufs=PIPE) as wpool,
            tc.tile_pool(name="pmm", bufs=PIPE, space="PSUM") as pmm,
        ):
            ones_rowb = cpool.tile([1, P], bf16)  # lhsT for x2 fold bcast
            nc.gpsimd.memset(ones_rowb, 1.0)
            # Issue order = global need order.  xr first (unblocks the
            # fold/warmup chain), then tile0's weights+data; xt is split by
            # contraction chunk so every packet is a contiguous 2KB
            # partition row.  Issues spread over Scalar/Sync/GpSimd queues.
            xr = cpool.tile([1, BL], bf16)        # x2 fold row (early, small)
            nc.scalar.dma_start(xr, xr_d)
            csv = cpool.tile([P, KT + 2], f32)   # cc 0..7 | scale | bias(t0)
            nc.scalar.dma_start(csv, cs_d)
            ct = bpool.tile([P, KT, KS, P], fp8, name="ct")
            ct_dv = ct_d.rearrange("p (t s j) -> p t s j", t=KT, s=KS)
            nc.sync.dma_start(ct[:, 0:1], ct_dv[:, 0:1])
            xt = bpool.tile([P, KS, BL], fp8, name="xt")
            xt_dv = xt_d.rearrange("p (s b) -> p s b", s=KS)
            nc.sync.dma_start(xt[:, 0:1], xt_dv[:, 0:1])
            nc.sync.dma_start(xt[:, 1:2], xt_dv[:, 1:2])
            nc.gpsimd.dma_start(ct[:, 1:KT], ct_dv[:, 1:KT])

            numbuf = bpool.tile([P, KT * BL], bf16)  # 32 KB/partition

            # ACT table prefetch: 1-elem ARS forces the lazy table load
            # during the DMA window instead of before the first real tile.
            tblw = cpool.tile([1, 1], f32)
            nc.scalar.activation(
                tblw, csv[0:1, 0:1], AF.Abs_reciprocal_sqrt, bias=0.0,
                scale=1.0,
            )
            nc.scalar.dma_start(xt[:, 2:3], xt_dv[:, 2:3])
            nc.scalar.dma_start(xt[:, 3:4], xt_dv[:, 3:4])

            def tile_mm(kt, ps=None):
                if ps is None:
                    ps = pmm.tile([P, BL], f32, tag="mm")
                for q in range(NQ):
                    psl = ps[:, q * 512 : (q + 1) * 512]
                    for s in range(KS // 2):
                        nc.tensor.matmul(
                            psl,
                            lhsT=ct[:, kt, 2 * s : 2 * s + 2, :],
                            rhs=xt[:, 2 * s : 2 * s + 2, q * 512 : (q + 1) * 512],
                            start=(s == 0),
                            stop=(s == KS // 2 - 1 and kt not in TFOLD),
                            perf_mode=DR,
                        )
                if kt in TFOLD:  # batched folds: one fp8->bf16 mode switch
                    for q in range(NQ):
                        nc.tensor.matmul(
                            ps[:, q * 512 : (q + 1) * 512],
                            lhsT=ones_rowb,
                            rhs=xr[0:1, q * 512 : (q + 1) * 512],
                            start=False,
                            stop=True,
                            skip_group_check=True,
                        )
                return ps

            def tile_fix(kt, ps):
                # w = (psum + cc_kt) + xx into SBUF, freeing psum for the PE
                w = wpool.tile([P, BL], f32, tag="w", name="w")
                nc.vector.scalar_tensor_tensor(
                    w, ps, csv[:, kt : kt + 1], xx, ALU.add, ALU.add
                )
                return w

            def tile_act(kt, src, bias):
                nc.scalar.activation(
                    numbuf[:, kt * BL : (kt + 1) * BL], src,
                    AF.Abs_reciprocal_sqrt,
                    bias=bias, scale=csv[:, KT : KT + 1],
                )

            def tile_store(kt):  # one k-tile per DMA (512 KB bf16)
                nc.sync.dma_start(
                    out_d[kt * P : (kt + 1) * P, :],
                    numbuf[:, kt * BL : (kt + 1) * BL],
                )

            # Materialize the x2 broadcast (128,BL) on device instead of
            # DMAing 512KB: 4 K=1 fold matmuls into tile1's psum bank (also
            # warms the PE during the input-DMA window), then one DVE copy
            # to SBUF.  tile_mm(1) later overwrites the bank with start=True.
            ps1 = pmm.tile([P, BL], f32, tag="mm")
            for q in range(NQ):
                nc.tensor.matmul(
                    ps1[:, q * 512 : (q + 1) * 512],
                    lhsT=ones_rowb,
                    rhs=xr[0:1, q * 512 : (q + 1) * 512],
                    start=True,
                    stop=True,
                    skip_group_check=True,
                )
            xx = bpool.tile([P, BL], f32, name="xx")
            nc.vector.tensor_scalar(
                xx, ps1, 0.0, 0.0, ALU.add, ALU.bypass
            )

            pend = {}
            pend[0] = tile_mm(0)
            pend[1] = tile_mm(1, ps1)
            for kt in range(KT):
                if kt + PIPE < KT:
                    pend[kt + PIPE] = tile_mm(kt + PIPE)
                ps = pend.pop(kt)
                if kt in TFOLD:
                    tile_act(kt, ps, csv[:, KT + 1 : KT + 2])
                else:
                    w = tile_fix(kt, ps)
                    tile_act(kt, w, 0.0)
                tile_store(kt)

    nc.finalize()
    return nc


def _get_bass():
    if "nc" not in _CACHE:
        _CACHE["nc"] = _build_bass()
    return _CACHE["nc"]


def _host_prep(x: np.ndarray, clusters: np.ndarray):
    import ml_dtypes

    x = np.ascontiguousarray(x, dtype=np.float32)
    c = np.ascontiguousarray(clusters, dtype=np.float32)
    x2 = np.einsum("bd,bd->b", x, x, optimize=True)          # (B,)
    c2 = np.einsum("kd,kd->k", c, c, optimize=True)          # (K,)
    x2m = float(x2.mean())

    # Normalizer estimate from a strided 2048-row sample (SE ~2e-4 rel).
    xs = x[::8]
    sq_s = x2[::8][:, None] - 2.0 * (xs @ c.T) + c2[None, :]
    dist_s = np.sqrt(np.maximum(sq_s, 0.0))
    S = float((1.0 / (1.0 + dist_s)).sum() * (B / xs.shape[0]))

    # Affine fit (1+sqrt(v))^2 ~ A*v + C, rel-weighted LSQ over padded range
    vmin = max(float(sq_s.min()) * 0.96, 1.0)
    vmax = float(sq_s.max()) * 1.04
    v = np.linspace(vmin, vmax, 4001)
    g = (1.0 + np.sqrt(v)) ** 2
    M = np.stack([v / g, 1.0 / g], axis=1)
    (A, C), *_ = np.linalg.lstsq(M, np.ones_like(v), rcond=None)
    h = (A * v + C) / g - 1.0
    corr = 1.0 + (h.max() + h.min()) / 2.0   # recenter the error band
    A, C = float(A / corr), float(C / corr)

    S2A = S * S * A
    # ACT input u = scale*w, w = psum + cc_p + xx_b, u = S2A*sq + S2*C:
    #   scale = -2*S2A; cc_p = -(c2_p + x2m + C/A)/2; xx_b = -(x2_b - x2m)/2
    # Tensor-fold tiles skip the DVE cc add; their ACT bias = scale*cc.
    cs = np.empty((P, KT + 2), dtype=np.float32)
    cs[:, :KT] = -(c2.reshape(KT, P).T + x2m + C / A) / 2.0
    cs[:, KT] = -2.0 * S2A
    cs[:, KT + 1] = cs[:, KT] * cs[:, 0]

    ct8 = np.ascontiguousarray(
        c.reshape(KT, P, KS, P).transpose(3, 0, 2, 1).reshape(P, KT * KS * P)
    ).astype(ml_dtypes.float8_e4m3)

    in_maps = []
    for ci in range(N_CORES):
        xc = x[ci * BL : (ci + 1) * BL]
        xt8 = np.ascontiguousarray(
            xc.T.reshape(KS, P, BL).transpose(1, 0, 2).reshape(P, KS * BL)
        ).astype(ml_dtypes.float8_e4m3)
        xxrow = (-(x2[ci * BL : (ci + 1) * BL] - x2m) / 2.0).astype(
            ml_dtypes.bfloat16
        )
        in_maps.append({
            "xt": xt8, "ct": ct8,
            "xr": np.ascontiguousarray(xxrow.reshape(1, BL)), "cs": cs,
        })
    return in_maps


def _assemble(results) -> np.ndarray:
    out = np.empty((B, K), dtype=np.float32)
    for ci, r in enumerate(results):
        slab = np.asarray(r["out"]).astype(np.float32)   # (K, BL)
        out[ci * BL : (ci + 1) * BL, :] = slab.T
    return out


def kernel(x: np.ndarray, clusters: np.ndarray) -> np.ndarray:
    from concourse.bass_utils import run_bass_kernel_spmd

    x = np.asarray(x, dtype=np.float32)
    clusters = np.asarray(clusters, dtype=np.float32)
    assert x.shape == (B, D) and clusters.shape == (K, D)

    in_maps = _host_prep(x, clusters)
    nc = _get_bass()
    res = run_bass_kernel_spmd(nc, in_maps, core_ids=list(range(N_CORES)))
    return _assemble(res.results)
